# revision 19
# baseline (speedup 1.0000x reference)
"""Trainium2 Bass kernel for CausalSelfAttention (B=4, T=2048, C=768, H=6, D=128)
with RoPE + QK-RMSNorm.  v3: bf16 datapath, single act-table, tile-pipelined.

Sharding: 8 cores = batch(4) x head-group(2, 3 heads each). Each core:
  - computes Q^T,K^T in (D, T) layout and V in (T, D) layout for its 3 heads
  - RoPE + RMSNorm on Q/K; rsqrt is computed as Exp(-0.5*Ln(x)) on the small
    (3,512) row-vectors so the whole kernel uses one activation table
    (natural_log_exp set: Exp/Ln/Copy) -> no table reloads
  - causal attention with scores computed transposed (S^T: T_k on partitions,
    T_q on free dim); heads emitted round-robin per k-chunk to hide exp latency
  - attention(qt=i) is emitted right after Q tile i is normed, with
    c_proj(i-1) as the PE filler in front of it -> PE rarely idles
  - partial c_proj over its 384 input channels; host sums the two head-group
    partials per batch
All SBUF tiles are bf16 (2x DVE, half DMA bytes); PSUM stays f32.
GPSIMD carries the diagonal tri-mask muls.
"""

import numpy as np
import ml_dtypes

_B, _T, _C, _H, _D = 4, 2048, 768, 6, 128
_HPG = 3            # heads per group
_HD = _HPG * _D     # 384, per-group head dims
_NT = 4             # T tiles of 512
_TW = 512           # tile width (T_q)
_NKC = _T // 128    # 16 k-chunks of 128
_NCB = _C // 128    # 6 c_in chunks
_EPS = 1e-15

_cached = {}


def _build_nc():
    from contextlib import ExitStack
    from concourse import bacc, tile, mybir

    f32 = mybir.dt.float32
    bf16 = mybir.dt.bfloat16
    Act = mybir.ActivationFunctionType

    nc = bacc.Bacc("TRN2", target_bir_lowering=False, debug=False)

    xT = nc.dram_tensor("xT", (128, _NCB * _T), bf16, kind="ExternalInput").ap()
    wq = nc.dram_tensor("wq", (128, _NCB * _HD), bf16, kind="ExternalInput").ap()
    wk = nc.dram_tensor("wk", (128, _NCB * _HD), bf16, kind="ExternalInput").ap()
    wv = nc.dram_tensor("wv", (128, _NCB * _HD), bf16, kind="ExternalInput").ap()
    wo = nc.dram_tensor("wo", (128, _HPG * _C), bf16, kind="ExternalInput").ap()
    cs = nc.dram_tensor("cs", (128, 2 * _T), bf16, kind="ExternalInput").ap()
    cst = nc.dram_tensor("cst", (128, 3 * 128), bf16, kind="ExternalInput").ap()
    out = nc.dram_tensor("out", (128, _NT * 4 * _C), bf16, kind="ExternalOutput").ap()

    with tile.TileContext(nc) as tc, ExitStack() as ctx, \
            nc.allow_low_precision(reason="bf16 datapath; f32 psum accumulation"):
        # --- pools ---
        pc = ctx.enter_context(tc.tile_pool(name="pc", bufs=1))         # persistents
        pg = ctx.enter_context(tc.tile_pool(name="pg", bufs=3))         # rope/sq scratch
        pa = ctx.enter_context(tc.tile_pool(name="pa", bufs=9))         # A chunks
        psm = ctx.enter_context(tc.tile_pool(name="psm", bufs=3))       # small vectors
        pob = ctx.enter_context(tc.tile_pool(name="pob", bufs=2))       # out staging
        # psum pools (8 banks): S/proj x3, O/sw x3, ms/den x1, bc/c_proj x1
        ppS = ctx.enter_context(tc.tile_pool(name="ppS", bufs=3, space="PSUM"))
        ppO = ctx.enter_context(tc.tile_pool(name="ppO", bufs=3, space="PSUM"))
        ppM = ctx.enter_context(tc.tile_pool(name="ppM", bufs=1, space="PSUM"))
        ppB = ctx.enter_context(tc.tile_pool(name="ppB", bufs=1, space="PSUM"))

        # --- DMA inputs (host pre-packs layouts; all bf16) ---
        # x chunk 0 first so the first K-proj matmul can start ASAP; weight
        # chunks interleaved with x chunks to pace the accumulation.
        t_xt = pc.tile([128, _NCB * _T], bf16, tag="xt", name="t_xt")
        t_wk = pc.tile([128, _NCB * _HD], bf16, tag="wk", name="t_wk")
        nc.sync.dma_start(t_xt[:, 0:_T], xT[:, 0:_T])
        nc.sync.dma_start(t_wk[:], wk[:])
        for c in range(1, _NCB):
            nc.sync.dma_start(t_xt[:, c * _T:(c + 1) * _T], xT[:, c * _T:(c + 1) * _T])
        t_cs = pc.tile([128, 2 * _T], bf16, tag="cs", name="t_cs")
        nc.sync.dma_start(t_cs[:], cs[:])
        t_cst = pc.tile([128, 3 * 128], bf16, tag="cst", name="t_cst")
        nc.sync.dma_start(t_cst[:], cst[:])
        t_wv = pc.tile([128, _NCB * _HD], bf16, tag="wv", name="t_wv")
        nc.sync.dma_start(t_wv[:], wv[:])
        t_wq = pc.tile([128, _NCB * _HD], bf16, tag="wq", name="t_wq")
        nc.sync.dma_start(t_wq[:], wq[:])
        t_wo = pc.tile([128, _HPG * _C], bf16, tag="wo", name="t_wo")
        nc.sync.dma_start(t_wo[:], wo[:])
        t_eps = pc.tile([128, 1], f32, tag="eps", name="t_eps")
        nc.gpsimd.memset(t_eps[:], _EPS)

        t_tri = t_cst[:, 0:128]
        t_ones_col = t_cst[:, 128:129]
        t_ones_row = t_cst[0:1, 128:256]
        t_perm = t_cst[:, 256:384]

        # persistent per-head K^T/Q^T (post rope+norm) and V blocks
        t_kn = [pc.tile([128, _T], bf16, tag=f"kn{h}", name=f"kn{h}") for h in range(_HPG)]
        t_qn = [pc.tile([128, _T], bf16, tag=f"qn{h}", name=f"qn{h}") for h in range(_HPG)]
        t_v = [pc.tile([128, _HD], bf16, tag=f"v{tb}", name=f"v{tb}") for tb in range(_NKC)]

        # ---------------- emitters ----------------
        def proj_qk(dst, w, i, h):
            """dst[:, i*512:(i+1)*512] = (W x)^T tile for head h."""
            p = ppS.tile([128, _TW], f32, tag="pS", name="p_qk")
            for c in range(_NCB):
                nc.tensor.matmul(
                    p[:], w[:, c * _HD + h * 128: c * _HD + (h + 1) * 128],
                    t_xt[:, c * _T + i * _TW: c * _T + (i + 1) * _TW],
                    start=(c == 0), stop=(c == _NCB - 1))
            nc.scalar.copy(dst[:, i * _TW:(i + 1) * _TW], p[:])

        def proj_v(tb):
            p = ppS.tile([128, _HD], f32, tag="pS", name="p_v",
                         padded_shape=[128, _TW])
            for c in range(_NCB):
                nc.tensor.matmul(
                    p[:], t_xt[:, c * _T + tb * 128: c * _T + (tb + 1) * 128],
                    t_wv[:, c * _HD:(c + 1) * _HD],
                    start=(c == 0), stop=(c == _NCB - 1))
            nc.scalar.copy(t_v[tb][:], p[:])

        def rope_stage(dst_list, i):
            """In-place RoPE on dst[h][:, i*512:+512] for all heads."""
            isl = slice(i * _TW, (i + 1) * _TW)
            ssl = slice(_T + i * _TW, _T + (i + 1) * _TW)
            sws = []
            for h in range(_HPG):
                p_sw = ppO.tile([128, _TW], f32, tag="pO", name="p_sw")
                nc.tensor.matmul(p_sw[:], t_perm, dst_list[h][:, isl],
                                 start=True, stop=True)
                t_sw = pg.tile([128, _TW], bf16, tag="sw", name="t_sw", bufs=3)
                nc.vector.tensor_mul(t_sw[:], p_sw[:], t_cs[:, ssl])
                sws.append(t_sw)
            for h in range(_HPG):
                nc.vector.tensor_mul(dst_list[h][:, isl], dst_list[h][:, isl],
                                     t_cs[:, isl])
                nc.vector.tensor_add(dst_list[h][:, isl], dst_list[h][:, isl],
                                     sws[h][:])

        def norm_stage(dst_list, i):
            """In-place RMSNorm over partitions (D). The 3 heads' mean-squares
            land in one (3,512) psum bank so Ln/Exp (rsqrt) runs once."""
            isl = slice(i * _TW, (i + 1) * _TW)
            sqs = []
            for h in range(_HPG):
                t_sq = pg.tile([128, _TW], bf16, tag="sq", name="t_sq", bufs=3)
                nc.vector.tensor_mul(t_sq[:], dst_list[h][:, isl],
                                     dst_list[h][:, isl])
                sqs.append(t_sq)
            # 3 heads' mean-squares land on partitions 0/32/64 of one psum
            # bank (matmul out base must be 0/32/64); Ln/Exp then run once on
            # the whole (65,512) strip -- unused rows hold unread garbage.
            p_ms = ppM.tile([65, _TW], f32, tag="pM", name="p_ms")
            for h in range(_HPG):
                nc.tensor.matmul(p_ms[32 * h:32 * h + 1, :], t_ones_col,
                                 sqs[h][:], start=True, stop=True)
            # rsqrt(ms/128+eps) = Exp(-0.5*Ln(ms/128+eps))
            t_ln = psm.tile([65, _TW], f32, tag="ln", name="t_ln", bufs=3)
            nc.scalar.activation(t_ln[:], p_ms[:], Act.Ln,
                                 bias=t_eps[0:65, :], scale=1.0 / 128.0)
            t_rs = psm.tile([65, _TW], bf16, tag="rs", name="t_rs", bufs=3)
            nc.scalar.activation(t_rs[:], t_ln[:], Act.Exp, scale=-0.5)
            for h in range(_HPG):
                p_bc = ppB.tile([128, _TW], f32, tag="pB", name="p_bc")
                nc.tensor.matmul(p_bc[:], t_cst[32 * h:32 * h + 1, 128:256],
                                 t_rs[32 * h:32 * h + 1, :],
                                 start=True, stop=True)
                nc.vector.tensor_mul(dst_list[h][:, isl], dst_list[h][:, isl],
                                     p_bc[:])

        def attention(qt):
            """Causal attention for all 3 heads of one T_q tile, head
            round-robin per k-chunk so exp latency hides behind the other
            heads' matmuls. Returns z tiles (unnormalized output * 1/den)."""
            nchunk = 4 * qt + 4
            LOOKAHEAD = 2
            p_den = ppM.tile([65, _TW], f32, tag="pM", name="p_den")
            p_os = [ppO.tile([128, _TW], f32, tag="pO", name=f"p_o{h}")
                    for h in range(_HPG)]
            a_tiles = {}

            def emit_s(kc, h):
                roff = 0 if kc < 4 * qt else (kc - 4 * qt) * 128
                nsl = slice(roff, _TW)
                ksl = slice(kc * 128, (kc + 1) * 128)
                p_s = ppS.tile([128, _TW], f32, tag="pS", name="p_s")
                nc.tensor.matmul(p_s[:, nsl], t_kn[h][:, ksl],
                                 t_qn[h][:, qt * _TW + roff:(qt + 1) * _TW],
                                 start=True, stop=True)
                t_a = pa.tile([128, _TW], bf16, tag="a", name="t_a")
                nc.scalar.activation(t_a[:, nsl], p_s[:, nsl], Act.Exp,
                                     scale=1.0 / float(np.sqrt(_D)))
                if kc >= 4 * qt:  # diagonal chunk: triangular mask
                    dsl = slice(roff, roff + 128)
                    nc.gpsimd.tensor_mul(t_a[:, dsl], t_a[:, dsl], t_tri)
                a_tiles[(kc, h)] = t_a

            def emit_acc(kc, h):
                roff = 0 if kc < 4 * qt else (kc - 4 * qt) * 128
                nsl = slice(roff, _TW)
                t_a = a_tiles.pop((kc, h))
                nc.tensor.matmul(p_den[32 * h:32 * h + 1, nsl], t_ones_col,
                                 t_a[:, nsl],
                                 start=(kc == 0), stop=(kc == nchunk - 1))
                nc.tensor.matmul(p_os[h][:, nsl],
                                 t_v[kc][:, h * 128:(h + 1) * 128], t_a[:, nsl],
                                 start=(kc == 0), stop=(kc == nchunk - 1))

            for kc in range(nchunk + LOOKAHEAD):
                for h in range(_HPG):
                    if kc < nchunk:
                        emit_s(kc, h)
                    if kc >= LOOKAHEAD:
                        emit_acc(kc - LOOKAHEAD, h)
            # normalization: one (3,512) den extraction, per-head bcast + mul
            t_dn = psm.tile([65, _TW], bf16, tag="dn", name="t_dn", bufs=3)
            nc.scalar.copy(t_dn[:], p_den[:])
            zs = []
            for h in range(_HPG):
                p_db = ppB.tile([128, _TW], f32, tag="pB", name="p_db")
                nc.tensor.matmul(p_db[:], t_cst[32 * h:32 * h + 1, 128:256],
                                 t_dn[32 * h:32 * h + 1, :],
                                 start=True, stop=True)
                t_db = pg.tile([128, _TW], bf16, tag="db", name="t_db", bufs=2)
                nc.vector.reciprocal(t_db[:], p_db[:])
                t_z = pc.tile([128, _TW], bf16, tag=f"z{h}_{qt % 2}",
                              name=f"z{h}_{qt % 2}")
                nc.vector.tensor_mul(t_z[:], p_os[h][:], t_db[:])
                zs.append(t_z)
            return zs

        def c_proj(qt, zs):
            t_ob = pob.tile([128, 4 * _C], bf16, tag="ob", name="t_ob")
            for blk in range(4):
                bsl = slice(blk * 128, (blk + 1) * 128)
                for nh in range(2):
                    p_c = ppB.tile([128, 384], f32, tag="pB", name="p_c",
                                   padded_shape=[128, _TW])
                    for hh in range(_HPG):
                        nc.tensor.matmul(
                            p_c[:], zs[hh][:, bsl],
                            t_wo[:, hh * _C + nh * 384: hh * _C + (nh + 1) * 384],
                            start=(hh == 0), stop=(hh == _HPG - 1))
                    o0 = blk * _C + nh * 384
                    nc.vector.tensor_copy(t_ob[:, o0:o0 + 384], p_c[:])
            nc.sync.dma_start(out[:, qt * 4 * _C:(qt + 1) * 4 * _C], t_ob[:])

        # ---------------- emission schedule (tile-pipelined) ----------------
        for i in range(_NT):
            for h in range(_HPG):
                proj_qk(t_kn[h], t_wk, i, h)
        prev = None  # (qt, zs) pending c_proj
        for i in range(_NT):
            rope_stage(t_kn, i)
            for tb in range(4 * i, 4 * i + 4):   # PE filler while DVE ropes K
                proj_v(tb)
            norm_stage(t_kn, i)
            for h in range(_HPG):                # PE filler while DVE norms K
                proj_qk(t_qn[h], t_wq, i, h)
            rope_stage(t_qn, i)
            norm_stage(t_qn, i)
            if prev is not None:                 # PE filler while DVE norms Q
                c_proj(*prev)
            prev = (i, attention(i))
        c_proj(*prev)

    nc.compile()
    return nc


def _get_nc():
    if "nc" not in _cached:
        _cached["nc"] = _build_nc()
    return _cached["nc"]


def _bf16(a):
    return np.ascontiguousarray(a.astype(ml_dtypes.bfloat16))


def make_in_maps(x, cos, sin, Wq, Wk, Wv, Wo):
    cosT = cos.reshape(_T, _D // 2).T                        # (64, T)
    sinT = sin.reshape(_T, _D // 2).T
    cc = np.concatenate([cosT, cosT], axis=0)                # (128, T)
    ss = np.concatenate([sinT, -sinT], axis=0)
    cs = _bf16(np.concatenate([cc, ss], axis=1))             # (128, 2T)
    tri = (np.arange(128)[None, :] >= np.arange(128)[:, None]).astype(np.float32)
    ones128 = np.ones((128, 128), dtype=np.float32)
    permm = np.zeros((128, 128), dtype=np.float32)           # half-swap permutation
    for d in range(64):
        permm[64 + d, d] = 1.0
        permm[d, 64 + d] = 1.0
    cst = _bf16(np.concatenate([tri, ones128, permm], axis=1))

    def pack_w(w):  # (768, 384) -> (128, 2304) c-chunk-major
        return _bf16(w.reshape(_NCB, 128, _HD).transpose(1, 0, 2).reshape(128, -1))

    def pack_wo(w):  # (384, 768) -> (128, 2304) head-chunk-major
        return _bf16(w.reshape(_HPG, 128, _C).transpose(1, 0, 2).reshape(128, -1))

    in_maps = []
    for core in range(8):
        b, g = divmod(core, 2)
        gsl = slice(g * _HD, (g + 1) * _HD)
        xt = x[b].T.reshape(_NCB, 128, _T).transpose(1, 0, 2).reshape(128, -1)
        in_maps.append({
            "xT": _bf16(xt),
            "wq": pack_w(Wq[gsl, :].T),
            "wk": pack_w(Wk[gsl, :].T),
            "wv": pack_w(Wv[gsl, :].T),
            "wo": pack_wo(Wo[:, gsl].T),
            "cs": cs, "cst": cst,
        })
    return in_maps


def unshard(core_outs):
    """core_outs: list of 8 per-core (128, 12288) bf16 arrays -> (4, 2048, 768)."""
    full = []
    for b in range(_B):
        acc = None
        for g in range(2):
            dev = np.asarray(core_outs[2 * b + g]).astype(np.float32)
            part = dev.reshape(128, _NT, 4, _C).transpose(1, 2, 0, 3).reshape(_T, _C)
            acc = part if acc is None else acc + part
        full.append(acc)
    return np.stack(full, axis=0)


def kernel(x, cos, sin, Wq, Wk, Wv, Wo):
    from concourse.bass_utils import run_bass_kernel_spmd

    x = np.asarray(x, dtype=np.float32)
    cos = np.asarray(cos, dtype=np.float32)
    sin = np.asarray(sin, dtype=np.float32)
    Wq = np.asarray(Wq, dtype=np.float32)
    Wk = np.asarray(Wk, dtype=np.float32)
    Wv = np.asarray(Wv, dtype=np.float32)
    Wo = np.asarray(Wo, dtype=np.float32)

    nc = _get_nc()
    in_maps = make_in_maps(x, cos, sin, Wq, Wk, Wv, Wo)
    res = run_bass_kernel_spmd(nc, in_maps, core_ids=list(range(8)))
    return unshard([r_["out"] for r_ in res.results])


# revision 21
# speedup vs baseline: 1.0786x; 1.0786x over previous
"""Trainium2 Bass kernel for CausalSelfAttention (B=4, T=2048, C=768, H=6, D=128)
with RoPE + QK-RMSNorm.  v3: bf16 datapath, single act-table, tile-pipelined.

Sharding: 8 cores = batch(4) x head-group(2, 3 heads each). Each core:
  - computes Q^T,K^T in (D, T) layout and V in (T, D) layout for its 3 heads
  - RoPE + RMSNorm on Q/K; rsqrt is computed as Exp(-0.5*Ln(x)) on the small
    (3,512) row-vectors so the whole kernel uses one activation table
    (natural_log_exp set: Exp/Ln/Copy) -> no table reloads
  - causal attention with scores computed transposed (S^T: T_k on partitions,
    T_q on free dim); heads emitted round-robin per k-chunk to hide exp latency
  - attention(qt=i) is emitted right after Q tile i is normed, with
    c_proj(i-1) as the PE filler in front of it -> PE rarely idles
  - partial c_proj over its 384 input channels; host sums the two head-group
    partials per batch
All SBUF tiles are bf16 (2x DVE, half DMA bytes); PSUM stays f32.
GPSIMD carries the diagonal tri-mask muls.
"""

import numpy as np
import ml_dtypes

_B, _T, _C, _H, _D = 4, 2048, 768, 6, 128
_HPG = 3            # heads per group
_HD = _HPG * _D     # 384, per-group head dims
_NT = 4             # T tiles of 512
_TW = 512           # tile width (T_q)
_NKC = _T // 128    # 16 k-chunks of 128
_NCB = _C // 128    # 6 c_in chunks
_EPS = 1e-15

_cached = {}


def _build_nc():
    from contextlib import ExitStack
    from concourse import bacc, tile, mybir

    f32 = mybir.dt.float32
    bf16 = mybir.dt.bfloat16
    Act = mybir.ActivationFunctionType

    nc = bacc.Bacc("TRN2", target_bir_lowering=False, debug=False)

    xT = nc.dram_tensor("xT", (128, _NCB * _T), bf16, kind="ExternalInput").ap()
    wq = nc.dram_tensor("wq", (128, _NCB * _HD), bf16, kind="ExternalInput").ap()
    wk = nc.dram_tensor("wk", (128, _NCB * _HD), bf16, kind="ExternalInput").ap()
    wv = nc.dram_tensor("wv", (128, _NCB * _HD), bf16, kind="ExternalInput").ap()
    wo = nc.dram_tensor("wo", (128, _HPG * _C), bf16, kind="ExternalInput").ap()
    cs = nc.dram_tensor("cs", (128, 2 * _T), bf16, kind="ExternalInput").ap()
    cst = nc.dram_tensor("cst", (128, 3 * 128), bf16, kind="ExternalInput").ap()
    out = nc.dram_tensor("out", (128, _NT * 4 * _C), bf16, kind="ExternalOutput").ap()

    with tile.TileContext(nc) as tc, ExitStack() as ctx, \
            nc.allow_low_precision(reason="bf16 datapath; f32 psum accumulation"):
        # --- pools ---
        pc = ctx.enter_context(tc.tile_pool(name="pc", bufs=1))         # persistents
        pg = ctx.enter_context(tc.tile_pool(name="pg", bufs=3))         # rope/sq scratch
        pa = ctx.enter_context(tc.tile_pool(name="pa", bufs=9))         # A chunks
        psm = ctx.enter_context(tc.tile_pool(name="psm", bufs=3))       # small vectors
        pob = ctx.enter_context(tc.tile_pool(name="pob", bufs=2))       # out staging
        # psum pools (8 banks): S/proj x3, O/sw x3, ms/den x1, bc/c_proj x1
        ppS = ctx.enter_context(tc.tile_pool(name="ppS", bufs=2, space="PSUM"))
        ppO = ctx.enter_context(tc.tile_pool(name="ppO", bufs=3, space="PSUM"))
        ppM = ctx.enter_context(tc.tile_pool(name="ppM", bufs=1, space="PSUM"))
        ppB = ctx.enter_context(tc.tile_pool(name="ppB", bufs=2, space="PSUM"))

        # --- DMA inputs (host pre-packs layouts; all bf16) ---
        # x chunk 0 first so the first K-proj matmul can start ASAP; weight
        # chunks interleaved with x chunks to pace the accumulation.
        t_xt = pc.tile([128, _NCB * _T], bf16, tag="xt", name="t_xt")
        t_wk = pc.tile([128, _NCB * _HD], bf16, tag="wk", name="t_wk")
        nc.sync.dma_start(t_xt[:, 0:_T], xT[:, 0:_T])
        nc.sync.dma_start(t_wk[:], wk[:])
        for c in range(1, _NCB):
            nc.sync.dma_start(t_xt[:, c * _T:(c + 1) * _T], xT[:, c * _T:(c + 1) * _T])
        t_cs = pc.tile([128, 2 * _T], bf16, tag="cs", name="t_cs")
        nc.sync.dma_start(t_cs[:], cs[:])
        t_cst = pc.tile([128, 3 * 128], bf16, tag="cst", name="t_cst")
        nc.sync.dma_start(t_cst[:], cst[:])
        t_wv = pc.tile([128, _NCB * _HD], bf16, tag="wv", name="t_wv")
        nc.sync.dma_start(t_wv[:], wv[:])
        t_wq = pc.tile([128, _NCB * _HD], bf16, tag="wq", name="t_wq")
        nc.sync.dma_start(t_wq[:], wq[:])
        t_wo = pc.tile([128, _HPG * _C], bf16, tag="wo", name="t_wo")
        nc.sync.dma_start(t_wo[:], wo[:])
        t_eps = pc.tile([128, 1], f32, tag="eps", name="t_eps")
        nc.gpsimd.memset(t_eps[:], _EPS)

        t_tri = t_cst[:, 0:128]
        t_ones_col = t_cst[:, 128:129]
        t_ones_row = t_cst[0:1, 128:256]
        t_perm = t_cst[:, 256:384]

        # persistent per-head K^T/Q^T (post rope+norm) and V blocks
        t_kn = [pc.tile([128, _T], bf16, tag=f"kn{h}", name=f"kn{h}") for h in range(_HPG)]
        t_qn = [pc.tile([128, _T], bf16, tag=f"qn{h}", name=f"qn{h}") for h in range(_HPG)]
        t_v = [pc.tile([128, _HD], bf16, tag=f"v{tb}", name=f"v{tb}") for tb in range(_NKC)]

        # ---------------- emitters ----------------
        def proj_qk(dst, w, i, h):
            """dst[:, i*512:(i+1)*512] = (W x)^T tile for head h."""
            p = ppS.tile([128, _TW], f32, tag="pS", name="p_qk")
            for c in range(_NCB):
                nc.tensor.matmul(
                    p[:], w[:, c * _HD + h * 128: c * _HD + (h + 1) * 128],
                    t_xt[:, c * _T + i * _TW: c * _T + (i + 1) * _TW],
                    start=(c == 0), stop=(c == _NCB - 1))
            nc.scalar.copy(dst[:, i * _TW:(i + 1) * _TW], p[:])

        def proj_v(tb):
            p = ppS.tile([128, _HD], f32, tag="pS", name="p_v",
                         padded_shape=[128, _TW])
            for c in range(_NCB):
                nc.tensor.matmul(
                    p[:], t_xt[:, c * _T + tb * 128: c * _T + (tb + 1) * 128],
                    t_wv[:, c * _HD:(c + 1) * _HD],
                    start=(c == 0), stop=(c == _NCB - 1))
            nc.scalar.copy(t_v[tb][:], p[:])

        def rope_stage(dst_list, i):
            """In-place RoPE on dst[h][:, i*512:+512] for all heads."""
            isl = slice(i * _TW, (i + 1) * _TW)
            ssl = slice(_T + i * _TW, _T + (i + 1) * _TW)
            sws = []
            for h in range(_HPG):
                p_sw = ppO.tile([128, _TW], f32, tag="pO", name="p_sw")
                nc.tensor.matmul(p_sw[:], t_perm, dst_list[h][:, isl],
                                 start=True, stop=True)
                t_sw = pg.tile([128, _TW], bf16, tag="sw", name="t_sw", bufs=3)
                nc.vector.tensor_mul(t_sw[:], p_sw[:], t_cs[:, ssl])
                sws.append(t_sw)
            for h in range(_HPG):
                nc.vector.tensor_mul(dst_list[h][:, isl], dst_list[h][:, isl],
                                     t_cs[:, isl])
                nc.vector.tensor_add(dst_list[h][:, isl], dst_list[h][:, isl],
                                     sws[h][:])

        def norm_stage(dst_list, i):
            """In-place RMSNorm over partitions (D). The 3 heads' mean-squares
            land in one (3,512) psum bank so Ln/Exp (rsqrt) runs once."""
            isl = slice(i * _TW, (i + 1) * _TW)
            sqs = []
            for h in range(_HPG):
                t_sq = pg.tile([128, _TW], bf16, tag="sq", name="t_sq", bufs=3)
                nc.vector.tensor_mul(t_sq[:], dst_list[h][:, isl],
                                     dst_list[h][:, isl])
                sqs.append(t_sq)
            # 3 heads' mean-squares land on partitions 0/32/64 of one psum
            # bank (matmul out base must be 0/32/64); Ln/Exp then run once on
            # the whole (65,512) strip -- unused rows hold unread garbage.
            p_ms = ppM.tile([65, _TW], f32, tag="pM", name="p_ms")
            for h in range(_HPG):
                nc.tensor.matmul(p_ms[32 * h:32 * h + 1, :], t_ones_col,
                                 sqs[h][:], start=True, stop=True)
            # rsqrt(ms/128+eps): one Act Sqrt + one DVE reciprocal per stage
            t_sd = psm.tile([65, _TW], f32, tag="sd", name="t_sd", bufs=3)
            nc.scalar.activation(t_sd[:], p_ms[:], Act.Sqrt,
                                 bias=t_eps[0:65, :], scale=1.0 / 128.0)
            t_rs = psm.tile([65, _TW], bf16, tag="rs", name="t_rs", bufs=3)
            nc.vector.reciprocal(t_rs[:], t_sd[:])
            for h in range(_HPG):
                p_bc = ppB.tile([128, _TW], f32, tag="pB", name="p_bc")
                nc.tensor.matmul(p_bc[:], t_cst[32 * h:32 * h + 1, 128:256],
                                 t_rs[32 * h:32 * h + 1, :],
                                 start=True, stop=True)
                nc.vector.tensor_mul(dst_list[h][:, isl], dst_list[h][:, isl],
                                     p_bc[:])

        def attention(qt):
            """Causal attention for all 3 heads of one T_q tile, head
            round-robin per k-chunk so exp latency hides behind the other
            heads' matmuls. Returns z tiles (unnormalized output * 1/den)."""
            nchunk = 4 * qt + 4
            LOOKAHEAD = 2
            p_den = ppM.tile([65, _TW], f32, tag="pM", name="p_den")
            p_os = [ppO.tile([128, _TW], f32, tag="pO", name=f"p_o{h}")
                    for h in range(_HPG)]
            a_tiles = {}

            def emit_s(kc, h):
                roff = 0 if kc < 4 * qt else (kc - 4 * qt) * 128
                nsl = slice(roff, _TW)
                ksl = slice(kc * 128, (kc + 1) * 128)
                p_s = ppS.tile([128, _TW], f32, tag="pS", name="p_s")
                nc.tensor.matmul(p_s[:, nsl], t_kn[h][:, ksl],
                                 t_qn[h][:, qt * _TW + roff:(qt + 1) * _TW],
                                 start=True, stop=True)
                t_a = pa.tile([128, _TW], bf16, tag="a", name="t_a")
                nc.scalar.activation(t_a[:, nsl], p_s[:, nsl], Act.Exp,
                                     scale=1.0 / float(np.sqrt(_D)))
                if kc >= 4 * qt:  # diagonal chunk: triangular mask
                    dsl = slice(roff, roff + 128)
                    nc.gpsimd.tensor_mul(t_a[:, dsl], t_a[:, dsl], t_tri)
                a_tiles[(kc, h)] = t_a

            def emit_acc(kc, h):
                roff = 0 if kc < 4 * qt else (kc - 4 * qt) * 128
                nsl = slice(roff, _TW)
                t_a = a_tiles.pop((kc, h))
                nc.tensor.matmul(p_den[32 * h:32 * h + 1, nsl], t_ones_col,
                                 t_a[:, nsl],
                                 start=(kc == 0), stop=(kc == nchunk - 1))
                nc.tensor.matmul(p_os[h][:, nsl],
                                 t_v[kc][:, h * 128:(h + 1) * 128], t_a[:, nsl],
                                 start=(kc == 0), stop=(kc == nchunk - 1))

            for kc in range(nchunk + LOOKAHEAD):
                for h in range(_HPG):
                    if kc < nchunk:
                        emit_s(kc, h)
                    if kc >= LOOKAHEAD:
                        emit_acc(kc - LOOKAHEAD, h)
            # normalization: one (3,512) den extraction, per-head bcast + mul
            t_dn = psm.tile([65, _TW], bf16, tag="dn", name="t_dn", bufs=3)
            nc.scalar.copy(t_dn[:], p_den[:])
            zs = []
            for h in range(_HPG):
                p_db = ppB.tile([128, _TW], f32, tag="pB", name="p_db")
                nc.tensor.matmul(p_db[:], t_cst[32 * h:32 * h + 1, 128:256],
                                 t_dn[32 * h:32 * h + 1, :],
                                 start=True, stop=True)
                t_db = pg.tile([128, _TW], bf16, tag="db", name="t_db", bufs=2)
                nc.vector.reciprocal(t_db[:], p_db[:])
                t_z = pc.tile([128, _TW], bf16, tag=f"z{h}_{qt % 2}",
                              name=f"z{h}_{qt % 2}")
                nc.vector.tensor_mul(t_z[:], p_os[h][:], t_db[:])
                zs.append(t_z)
            return zs

        def c_proj(qt, zs):
            t_ob = pob.tile([128, 4 * _C], bf16, tag="ob", name="t_ob")
            for blk in range(4):
                bsl = slice(blk * 128, (blk + 1) * 128)
                for nh in range(2):
                    p_c = ppB.tile([128, 384], f32, tag="pB", name="p_c",
                                   padded_shape=[128, _TW])
                    for hh in range(_HPG):
                        nc.tensor.matmul(
                            p_c[:], zs[hh][:, bsl],
                            t_wo[:, hh * _C + nh * 384: hh * _C + (nh + 1) * 384],
                            start=(hh == 0), stop=(hh == _HPG - 1))
                    o0 = blk * _C + nh * 384
                    nc.vector.tensor_copy(t_ob[:, o0:o0 + 384], p_c[:])
            nc.sync.dma_start(out[:, qt * 4 * _C:(qt + 1) * 4 * _C], t_ob[:])

        # ---------------- emission schedule (tile-pipelined) ----------------
        for i in range(_NT):
            for h in range(_HPG):
                proj_qk(t_kn[h], t_wk, i, h)
        prev = None  # (qt, zs) pending c_proj
        for i in range(_NT):
            rope_stage(t_kn, i)
            for tb in range(4 * i, 4 * i + 4):   # PE filler while DVE ropes K
                proj_v(tb)
            norm_stage(t_kn, i)
            for h in range(_HPG):                # PE filler while DVE norms K
                proj_qk(t_qn[h], t_wq, i, h)
            rope_stage(t_qn, i)
            norm_stage(t_qn, i)
            if prev is not None:                 # PE filler while DVE norms Q
                c_proj(*prev)
            prev = (i, attention(i))
        c_proj(*prev)

    nc.compile()
    return nc


def _get_nc():
    if "nc" not in _cached:
        _cached["nc"] = _build_nc()
    return _cached["nc"]


def _bf16(a):
    return np.ascontiguousarray(a.astype(ml_dtypes.bfloat16))


def make_in_maps(x, cos, sin, Wq, Wk, Wv, Wo):
    cosT = cos.reshape(_T, _D // 2).T                        # (64, T)
    sinT = sin.reshape(_T, _D // 2).T
    cc = np.concatenate([cosT, cosT], axis=0)                # (128, T)
    ss = np.concatenate([sinT, -sinT], axis=0)
    cs = _bf16(np.concatenate([cc, ss], axis=1))             # (128, 2T)
    tri = (np.arange(128)[None, :] >= np.arange(128)[:, None]).astype(np.float32)
    ones128 = np.ones((128, 128), dtype=np.float32)
    permm = np.zeros((128, 128), dtype=np.float32)           # half-swap permutation
    for d in range(64):
        permm[64 + d, d] = 1.0
        permm[d, 64 + d] = 1.0
    cst = _bf16(np.concatenate([tri, ones128, permm], axis=1))

    def pack_w(w):  # (768, 384) -> (128, 2304) c-chunk-major
        return _bf16(w.reshape(_NCB, 128, _HD).transpose(1, 0, 2).reshape(128, -1))

    def pack_wo(w):  # (384, 768) -> (128, 2304) head-chunk-major
        return _bf16(w.reshape(_HPG, 128, _C).transpose(1, 0, 2).reshape(128, -1))

    in_maps = []
    for core in range(8):
        b, g = divmod(core, 2)
        gsl = slice(g * _HD, (g + 1) * _HD)
        xt = x[b].T.reshape(_NCB, 128, _T).transpose(1, 0, 2).reshape(128, -1)
        in_maps.append({
            "xT": _bf16(xt),
            "wq": pack_w(Wq[gsl, :].T),
            "wk": pack_w(Wk[gsl, :].T),
            "wv": pack_w(Wv[gsl, :].T),
            "wo": pack_wo(Wo[:, gsl].T),
            "cs": cs, "cst": cst,
        })
    return in_maps


def unshard(core_outs):
    """core_outs: list of 8 per-core (128, 12288) bf16 arrays -> (4, 2048, 768)."""
    full = []
    for b in range(_B):
        acc = None
        for g in range(2):
            dev = np.asarray(core_outs[2 * b + g]).astype(np.float32)
            part = dev.reshape(128, _NT, 4, _C).transpose(1, 2, 0, 3).reshape(_T, _C)
            acc = part if acc is None else acc + part
        full.append(acc)
    return np.stack(full, axis=0)


def kernel(x, cos, sin, Wq, Wk, Wv, Wo):
    from concourse.bass_utils import run_bass_kernel_spmd

    x = np.asarray(x, dtype=np.float32)
    cos = np.asarray(cos, dtype=np.float32)
    sin = np.asarray(sin, dtype=np.float32)
    Wq = np.asarray(Wq, dtype=np.float32)
    Wk = np.asarray(Wk, dtype=np.float32)
    Wv = np.asarray(Wv, dtype=np.float32)
    Wo = np.asarray(Wo, dtype=np.float32)

    nc = _get_nc()
    in_maps = make_in_maps(x, cos, sin, Wq, Wk, Wv, Wo)
    res = run_bass_kernel_spmd(nc, in_maps, core_ids=list(range(8)))
    return unshard([r_["out"] for r_ in res.results])
